# revision 11
# baseline (speedup 1.0000x reference)
"""NF5 blockwise fake-quantized embedding lookup on 8 TRN2 NeuronCores.

Strategy (data-parallel): the [8,4096] int32 index tensor is flattened and
split into 8 shards of 4096 rows; each core holds the full fp32 table in
DRAM, gathers its 4096 rows via indirect DMA, applies the blockwise (64-elem)
NF5 fake quantization on-chip, and writes its [4096,1024] slice out.

Quantization pipeline per [128,1024] tile (16 blocks of 64 per row):
  m1   = per-block max|x|                 (DVE reduce, abs)
  m2   = per-block 2nd max|x|             (custom DVE mask + DVE reduce)
  scale= m2*lw + m1*hw (fp32, == jnp.quantile(|x|,0.999) linear interp)
  q    = x * (1/scale)                    (GPSIMD)
  e    = erf(C*q)                         (ACT)   -- smooth bin coordinate
  k0   = floor(A*e + 15.5)  via bf16 round-trick  (DVE stock, 4x)
  H    = exact +-1 bin correction: compare e against reconstructed
         boundary erf(C*mid[k0]) via a monic cubic in ln(1-m^2)   (ACT+DVE)
  w    = (2(k0+H)-31)/32, clamped to +-31/32
  deq  = P(ln(1-w^2))*w * scale  with P a monic cubic calibrated so that
         deq hits the 32 NF5 levels exactly (1.4e-5)              (ACT+DVE)
"""

import numpy as np

# ---------------------------------------------------------------- constants
P = 128
D = 1024
S = 16          # blocks per row
BS = 64         # block size
V = 50257       # vocab
B, SEQ = 8, 4096
N_CORES = 8
ROWS_PER_CORE = B * SEQ // N_CORES   # 4096
N_TILES = ROWS_PER_CORE // P         # 32

ERF_A = 15.848579370497898
ERF_C = 1.5411323548844962
HW32 = np.float32(np.float32(0.999) * np.float32(63.0) - np.float32(62.0))
LW32 = np.float32(np.float32(1.0) - HW32)

# boundary reconstruction: erf(C*mid[k]) ~= c3x*(((L+x2)L+x1)L+x0)*m,
# m=(k-15)/16, L=ln(1-m^2)
X2 = -1.6020923457840899
X1 = -9.761162437471903
X0 = -1934.1638692416411
C3X = -0.0005235416180417601
GAMMA = 1.0 / C3X  # e-prescale; C3X<0 flips the compare to is_lt

# level reconstruction: lv[k] ~= c3a*(((L+a2)L+a1)L+a0)*w, w=(2k-31)/32
A2 = 5.0452874329053135
A1 = -109.66810724396461
A0 = 417.6402356073725
C3A = 0.0013930804843840787

_CACHE = {}


def _split_sync_waits(nc, mybir, lim=1):
    """This walrus build rejects more than ~1 sem wait per instruction.
    Move excess waits onto same-engine NoOps inserted just before the
    waiting instruction."""
    n_split = 0
    for f in nc.m.functions:
        for bb in f.blocks:
            new = []
            for ins in bb.instructions:
                si = ins.sync_info
                waits = list(si.on_wait) if si and si.on_wait else []
                if len(waits) > lim:
                    n_split += 1
                    for i, w in enumerate(waits[lim:]):
                        nop = mybir.InstNoOp(
                            name=f"{ins.name}-sw{i}",
                            sync_info=mybir.SyncInfo(on_wait=[w], on_update=[]),
                            bass_nofuse=True,
                            engine=ins.engine,
                        )
                        new.append(nop)
                    si.on_wait = waits[:lim]
                new.append(ins)
            bb.instructions[:] = new
    return n_split


def _register_dve_ops():
    """Register the custom DVE ops (runtime registration; shas computed)."""
    import concourse.dve_ops as dvo
    from concourse.dve_spec import (
        Spec, Src0, Src1, Zero, C0, C1, C2, MaxNeg, maxx, select, lower,
    )
    from concourse.dve_uop import DveOpSpec

    if "NF5_M2MASK" in dvo._SUB_OPCODE_FOR_NAME:
        return (
            dvo.CUSTOM_DVE_SPECS and
            {op.name: op for op in dvo.OPS}["NF5_M2MASK"],
            {op.name: op for op in dvo.OPS}["NF5_MONIC3"],
        )

    def _mk(name, spec):
        shas = {}
        for ver in ("v3", "v4"):
            uops = lower(spec, ver=ver)
            from concourse.dve_spec import _has_src1
            tmp = DveOpSpec(name=name, opcode=0, uops=uops,
                            rd1_en=_has_src1(spec))
            shas[ver] = tmp.sha(ver)
        op = dvo.DveOp(name, spec, subdim=False, uops_sha=shas)
        dvo.OPS.append(op)
        dvo.CUSTOM_DVE_SPECS[name] = spec
        dvo._SUB_OPCODE_FOR_NAME[name] = (
            dvo._CUSTOM_DVE_ROW_BASE + len(dvo.OPS) - 1
        )
        assert dvo._SUB_OPCODE_FOR_NAME[name] < 0x20
        return op

    # masked abs: out = (|x| >= m1) ? 0 : |x| ; in1 = m1 page-broadcast
    absx = maxx(Src0, Zero - Src0)
    m2mask = _mk(
        "NF5_M2MASK",
        Spec(
            body=select(absx >= Src1, Zero, absx),
            reference=lambda in0, in1, s0, s1, imm2: np.where(
                np.abs(in0) >= in1, 0.0, np.abs(in0)
            ).astype(np.float32),
        ),
    )

    # monic cubic times stream: out = (((L+s0)L+s1)L+imm2) * in1
    monic3 = _mk(
        "NF5_MONIC3",
        Spec(
            body=(((Src0 + C0) * Src0 + C1) * Src0 + C2) * Src1,
            reference=lambda in0, in1, s0, s1, imm2: (
                ((((in0.astype(np.float64) + s0) * in0 + s1) * in0 + imm2)
                 * in1).astype(np.float32)
            ),
        ),
    )
    return m2mask, monic3


DEFAULT_CFG = dict(
    rows_tile=128,      # rows per compute tile (multiple of 128)
    bufs_x=4, bufs_mid=3, bufs_y=4,
    share_tags=False,
    norm_eng="gp",      # q = x*rinv: gp | dve
    e2_eng="act",       # e2 = e*gamma: act | dve
    isgt_eng="dve",     # H compare: dve | gp
    pout_eng="alt",     # final scale mult: dve | gp | alt (alternate per tile)
    sqm_eng="act",      # u_m = m^2: act | dve
    sqw_eng="act",      # u_w = wc^2: act | dve
)


def _build_module(cfg=None):
    import concourse.bass as bass
    import concourse.bacc as bacc
    import concourse.mybir as mybir
    import concourse.tile as tile

    cfg = dict(DEFAULT_CFG, **(cfg or {}))
    sh = cfg["share_tags"]
    tg = (lambda a, b: b) if sh else (lambda a, b: a)
    M2MASK, MONIC3 = _register_dve_ops()

    f32 = mybir.dt.float32
    bf16 = mybir.dt.bfloat16
    i32 = mybir.dt.int32
    Alu = mybir.AluOpType
    Act = mybir.ActivationFunctionType

    nc = bacc.Bacc(
        "TRN2",
        target_bir_lowering=False,
        debug=False,
        enable_asserts=False,
        num_devices=N_CORES,
    )
    idx_d = nc.dram_tensor("idx", [ROWS_PER_CORE, 1], i32, kind="ExternalInput")
    w_d = nc.dram_tensor("w", [V, D], f32, kind="ExternalInput")
    out_d = nc.dram_tensor("out", [ROWS_PER_CORE, D], f32, kind="ExternalOutput")

    def b3(ap_2d, ts_):
        # [128, TS] -> [128, TS, BS] broadcast (step-0 inner)
        return ap_2d.unsqueeze(2).to_broadcast([P, ts_, BS])

    B_X, B_M, B_Y = cfg["bufs_x"], cfg["bufs_mid"], cfg["bufs_y"]
    RT = cfg["rows_tile"]
    RPP = RT // P            # rows per partition per tile
    TD = RPP * D             # free elems per partition per tile
    TS = RPP * S             # blocks per partition per tile
    n_tiles = ROWS_PER_CORE // RT
    with tile.TileContext(nc) as tc:
        with tc.tile_pool(name="x", bufs=B_X) as px, \
             tc.tile_pool(name="scr", bufs=B_M) as pscr, \
             tc.tile_pool(name="st", bufs=B_X) as pst, \
             tc.tile_pool(name="q", bufs=B_M) as pq, \
             tc.tile_pool(name="e", bufs=B_M) as pe, \
             tc.tile_pool(name="sm", bufs=B_M) as psm, \
             tc.tile_pool(name="big", bufs=B_M) as pbig, \
             tc.tile_pool(name="y", bufs=B_Y) as py, \
             tc.tile_pool(name="idx", bufs=B_X) as pidx:
            n_groups = ROWS_PER_CORE // P
            idx_all = pidx.tile([P, n_groups], i32)
            nc.sync.dma_start(
                idx_all[:],
                idx_d[:, :].rearrange("(g p) o -> p (g o)", p=P),
            )
            for t in range(n_tiles):
                x = px.tile([P, TD], f32)
                for j in range(RPP):
                    g = t * RPP + j
                    nc.gpsimd.indirect_dma_start(
                        out=x[:, j * D:(j + 1) * D],
                        out_offset=None,
                        in_=w_d[:, :],
                        in_offset=bass.IndirectOffsetOnAxis(
                            ap=idx_all[:, g:g + 1], axis=0),
                    )
                x3 = x[:].rearrange("p (s n) -> p s n", n=BS)

                # --- stats ---
                m1 = pst.tile([P, TS], f32, tag="m1")
                nc.vector.tensor_reduce(
                    m1[:], x3, axis=mybir.AxisListType.X, op=Alu.max,
                    apply_absolute_value=True,
                )
                scr = pscr.tile([P, TD], f32)
                scr3 = scr[:].rearrange("p (s n) -> p s n", n=BS)
                nc.vector._custom_dve(M2MASK, out=scr3, in0=x3, in1=b3(m1[:], TS))
                m2 = pst.tile([P, TS], f32, tag="m2")
                nc.vector.tensor_reduce(
                    m2[:], scr3, axis=mybir.AxisListType.X, op=Alu.max,
                )
                tl = pst.tile([P, TS], f32, tag="tl")
                nc.vector.tensor_scalar(tl[:], m2[:], float(LW32), None, Alu.mult)
                sc = pst.tile([P, TS], f32, tag="sc")
                nc.vector.scalar_tensor_tensor(
                    sc[:], m1[:], float(HW32), tl[:], Alu.mult, Alu.add
                )
                smax = pst.tile([P, TS], f32, tag="smax")
                nc.vector.tensor_scalar(smax[:], sc[:], 1e-8, None, Alu.max)
                rinv = pst.tile([P, TS], f32, tag="rinv")
                nc.vector.reciprocal(rinv[:], smax[:])
                smax3 = pst.tile([P, TS], f32, tag="smax3")
                nc.vector.tensor_scalar(smax3[:], smax[:], C3A, None, Alu.mult)

                # --- normalize (GPSIMD) ---
                q = pq.tile([P, TD], f32)
                q3 = q[:].rearrange("p (s n) -> p s n", n=BS)
                norm_eng = nc.gpsimd if cfg["norm_eng"] == "gp" else nc.vector
                norm_eng.tensor_tensor(q3, x3, b3(rinv[:], TS), Alu.mult)

                # --- smooth bin coordinate ---
                e = pe.tile([P, TD], f32, tag="e")
                nc.scalar.activation(e[:], q[:], Act.Erf, bias=0.0, scale=ERF_C)
                r = psm.tile([P, TD], bf16, tag="r")
                nc.vector.tensor_scalar(
                    r[:], e[:], ERF_A, 144.0, Alu.mult, Alu.add
                )  # bf16 write rounds -> r = 129 + floor(t)
                w0 = psm.tile([P, TD], bf16, tag="w0")
                nc.vector.tensor_scalar(
                    w0[:], r[:], 144.5, 0.0625, Alu.subtract, Alu.mult
                )
                mm = psm.tile([P, TD], bf16, tag="mm")
                nc.vector.tensor_scalar(
                    mm[:], r[:], 144.0, 0.0625, Alu.subtract, Alu.mult
                )

                # --- exact +-1 correction ---
                um = pbig.tile([P, TD], f32, tag=tg("um", "usq"))
                if cfg["sqm_eng"] == "act":
                    nc.scalar.activation(um[:], mm[:], Act.Square)
                else:
                    nc.vector.tensor_tensor(um[:], mm[:], mm[:], Alu.mult)
                lm = pbig.tile([P, TD], f32, tag=tg("lm", "lln"))
                nc.scalar.activation(lm[:], um[:], Act.Ln, bias=1.0, scale=-1.0)
                xim = pbig.tile([P, TD], f32, tag=tg("xim", "poly"))
                nc.vector._custom_dve(
                    MONIC3, out=xim[:], in0=lm[:], in1=mm[:],
                    s0=X2, s1=X1, imm2=X0,
                )
                e2 = pe.tile([P, TD], f32, tag="e2")
                if cfg["e2_eng"] == "act":
                    nc.scalar.activation(e2[:], e[:], Act.Copy, bias=0.0,
                                         scale=GAMMA)
                else:
                    nc.vector.tensor_scalar(e2[:], e[:], GAMMA, None, Alu.mult)
                hh = psm.tile([P, TD], bf16, tag="hh")
                isgt_eng = nc.gpsimd if cfg["isgt_eng"] == "gp" else nc.vector
                isgt_eng.tensor_tensor(hh[:], e2[:], xim[:], Alu.is_lt)
                w1 = psm.tile([P, TD], bf16, tag="w1")
                nc.vector.scalar_tensor_tensor(
                    w1[:], hh[:], 0.0625, w0[:], Alu.mult, Alu.add
                )
                wc = psm.tile([P, TD], bf16, tag="wc")
                nc.vector.tensor_scalar(
                    wc[:], w1[:], -0.96875, 0.96875, Alu.max, Alu.min
                )

                # --- level reconstruction ---
                uw = pbig.tile([P, TD], f32, tag=tg("uw", "usq"))
                if cfg["sqw_eng"] == "act":
                    nc.scalar.activation(uw[:], wc[:], Act.Square)
                else:
                    nc.vector.tensor_tensor(uw[:], wc[:], wc[:], Alu.mult)
                lw = pbig.tile([P, TD], f32, tag=tg("lw", "lln"))
                nc.scalar.activation(lw[:], uw[:], Act.Ln, bias=1.0, scale=-1.0)
                aa = pbig.tile([P, TD], f32, tag=tg("aa", "poly"))
                nc.vector._custom_dve(
                    MONIC3, out=aa[:], in0=lw[:], in1=wc[:],
                    s0=A2, s1=A1, imm2=A0,
                )
                y = py.tile([P, TD], f32)
                y3 = y[:].rearrange("p (s n) -> p s n", n=BS)
                a3 = aa[:].rearrange("p (s n) -> p s n", n=BS)
                pe_ = cfg["pout_eng"]
                if pe_ == "alt":
                    pout_eng = nc.gpsimd if (t % 2 == 0) else nc.vector
                elif pe_ == "gp":
                    pout_eng = nc.gpsimd
                else:
                    pout_eng = nc.vector
                pout_eng.tensor_tensor(y3, a3, b3(smax3[:], TS), Alu.mult)
                for j in range(RPP):
                    rows = slice(t * RT + j * P, t * RT + (j + 1) * P)
                    nc.sync.dma_start(out_d[rows, :], y[:, j * D:(j + 1) * D])
    nc.compile()
    return nc


def _get_module(cfg=None):
    key = tuple(sorted((dict(DEFAULT_CFG, **(cfg or {}))).items()))
    if key not in _CACHE:
        _CACHE[key] = _build_module(cfg)
    return _CACHE[key]


def run(input, weight, trace=False, trace_kwargs=None):
    from concourse.bass_utils import run_bass_kernel_spmd

    nc = _get_module()
    idx_flat = np.ascontiguousarray(
        np.asarray(input, dtype=np.int32).reshape(-1, 1)
    )
    w = np.ascontiguousarray(np.asarray(weight, dtype=np.float32))
    in_maps = [
        {
            "idx": idx_flat[c * ROWS_PER_CORE:(c + 1) * ROWS_PER_CORE],
            "w": w,
        }
        for c in range(N_CORES)
    ]
    res = run_bass_kernel_spmd(
        nc, in_maps, core_ids=list(range(N_CORES)), trace=trace,
        trace_kwargs=trace_kwargs or {},
    )
    out = np.concatenate([res.results[c]["out"] for c in range(N_CORES)], axis=0)
    return out.reshape(B, SEQ, D), res


def kernel(input, weight):
    out, _ = run(input, weight, trace=False)
    return out
